# revision 6
# baseline (speedup 1.0000x reference)
"""ANOVA-kernel (order 3) Trainium2 Bass kernel.

Reference computes, per batch b: sum_d e3(x[b, :, d]) where e3 is the 3rd
elementary symmetric polynomial over the F=64 fields. Newton's identities:

    e3 = (p1^3 - 3 p1 p2 + 2 p3) / 6,   p_k[b, d] = sum_f x[b, f, d]^k

so the sequential DP scan becomes power-sum reductions.

The kernel streams x as fp16 (host-side cast; quantization contributes
~1.5e-3 norm-rel vs the 2e-2 tolerance), halving HBM traffic and enabling
the DVE 16-bit 2x mode. The input keeps its native [b, F, D] layout with
d contiguous, so every pairwise f-fold is an add of two CONTIGUOUS tile
halves: 2x-packed on DVE, bulk-transferable by DMA.

Per [128 x 4096] tile (batch on partitions, free = (f, d)):
  - p1: first f-fold runs on the DMA engines (software-DGE copy + accum
    add — the bus has ~50% slack at fp16), then two 2x folds + one small
    grouped reduce on DVE.
  - x^2 (fp16): Scalar-engine Square or DVE tensor_mul (2x), per tile.
  - p2: first fold of x^2 on GPSIMD (otherwise idle), then the same DVE
    tail.
  - sum_d p3 via the sin trick: sum sin(t x) = t P1 - t^3 P3/6 + O(t^5);
    one Scalar pass per tile, dead output to PSUM (saves SBUF write
    bandwidth), free per-partition accumulate.
  - epilogue: out = sum_d (1/6) p1 (p1^2 - 3 p2) + 128 P1 - 1024 S1.

Sharding: pure data parallel over the batch dim across 8 NeuronCores.
"""

import numpy as np

_B, _F, _D = 8192, 64, 64
_NCORES = 8
_BP = _B // _NCORES  # batches per core
_P = 128             # partitions per tile
_FD = _F * _D        # free elems per batch

# square engine per tile index: 'a' = ACT, 'd' = DVE, 'g' = GPSIMD
_SQ_ENGINE = "dddadada"
# p1 first fold on the DMA engines (software DGE accum); else DVE
_DMA_FOLD = True


def build_nc(bp=_BP, sq_engine=_SQ_ENGINE, dma_fold=_DMA_FOLD):
    """Build the per-core Bass graph for bp batches.

    Inputs:  "x"   [bp, 64, 64] f16 in native (b, f, d) layout
    Outputs: "out" [128, bp/128] f32 with out[p, t] = y[t*128 + p]
    """
    from contextlib import ExitStack

    from concourse import bacc, mybir, tile

    f32 = mybir.dt.float32
    f16 = mybir.dt.float16
    AF = mybir.ActivationFunctionType
    OP = mybir.AluOpType
    AX = mybir.AxisListType

    T = bp // _P  # tiles per core
    assert bp % _P == 0
    H = _FD // 2

    nc = bacc.Bacc("TRN2", target_bir_lowering=False, debug=False)
    x_ext = nc.dram_tensor("x", [bp, _F, _D], f16, kind="ExternalInput").ap()
    y_ext = nc.dram_tensor("out", [_P, T], f32, kind="ExternalOutput").ap()

    with tile.TileContext(nc) as tc, ExitStack() as ctx:
        xp = ctx.enter_context(tc.tile_pool(name="xt", bufs=T))
        sqp = ctx.enter_context(tc.tile_pool(name="sq", bufs=3))
        fap = ctx.enter_context(tc.tile_pool(name="fa", bufs=3))
        fbp = ctx.enter_context(tc.tile_pool(name="fb", bufs=2))
        fcp = ctx.enter_context(tc.tile_pool(name="fc", bufs=2))
        gap = ctx.enter_context(tc.tile_pool(name="ga", bufs=3))
        gbp = ctx.enter_context(tc.tile_pool(name="gb", bufs=2))
        gcp = ctx.enter_context(tc.tile_pool(name="gc", bufs=2))
        pers = ctx.enter_context(tc.tile_pool(name="pers", bufs=1))

        p1b = pers.tile([_P, T * _D], f32, tag="p1b")
        p2b = pers.tile([_P, T * _D], f32, tag="p2b")
        sa1 = pers.tile([_P, T], f32, tag="sa1")
        eacc = pers.tile([_P, T], f32, tag="eacc")
        p1f = pers.tile([_P, T], f32, tag="p1f")
        dq = pers.tile([_P, T], f32, tag="dq")
        out8 = pers.tile([_P, T], f32, tag="out8")
        sindead = pers.tile([_P, _FD], f16, tag="sindead")
        r_ = pers.tile([_P, 6 * _D], f32, tag="r_")
        z_ = pers.tile([_P, 6 * _D], f32, tag="z_")
        w_ = pers.tile([_P, 6 * _D], f32, tag="w_")

        xv_dram = x_ext.rearrange("(t p) f d -> t p (f d)", p=_P)

        # warm the Sin activation table during the initial DMA wait
        warm = pers.tile([_P, 1], f32, tag="warm")
        nc.gpsimd.memset(warm[:], 0.0)
        nc.scalar.activation(warm[:], warm[:], AF.Sin, scale=0.125)

        def chain_tail(l1, big, k):
            """Two 2x folds + grouped f-reduce: l1 [P, 2048] -> big cols k."""
            fb = fbp.tile([_P, _FD // 4], f16, tag="fb")
            nc.vector.tensor_add(fb[:], l1[:, :_FD // 4], l1[:, _FD // 4:])
            fc = fcp.tile([_P, _FD // 8], f16, tag="fc")
            nc.vector.tensor_add(fc[:], fb[:, :_FD // 8], fb[:, _FD // 8:])
            # fc layout (f=8, d=64); reduce over strided f
            nc.vector.reduce_sum(
                big[:, k * _D:(k + 1) * _D],
                fc[:].rearrange("p (f d) -> p d f", f=8),
                axis=AX.X,
            )

        def chain_tail2(l1, big, k, bp_, cp_):
            fb = bp_.tile([_P, _FD // 4], f16, tag="gb")
            nc.vector.tensor_add(fb[:], l1[:, :_FD // 4], l1[:, _FD // 4:])
            fc = cp_.tile([_P, _FD // 8], f16, tag="gc")
            nc.vector.tensor_add(fc[:], fb[:, :_FD // 8], fb[:, _FD // 8:])
            nc.vector.reduce_sum(
                big[:, k * _D:(k + 1) * _D],
                fc[:].rearrange("p (f d) -> p d f", f=8),
                axis=AX.X,
            )

        def emit_epi(k0, k1):
            """eacc = sum_d (1/6) p1 (p1^2 - 3 p2); p1f = sum_d p1."""
            n = (k1 - k0) * _D
            p1s = p1b[:, k0 * _D:k1 * _D]
            p2s = p2b[:, k0 * _D:k1 * _D]
            nc.vector.tensor_mul(r_[:, :n], p1s, p1s)
            nc.vector.scalar_tensor_tensor(
                z_[:, :n], p2s, -3.0, r_[:, :n], OP.mult, OP.add
            )
            nc.vector.scalar_tensor_tensor(
                w_[:, :n], p1s, 1.0 / 6.0, z_[:, :n], OP.mult, OP.mult
            )
            nc.vector.reduce_sum(
                eacc[:, k0:k1],
                w_[:, :n].rearrange("p (t d) -> p t d", d=_D),
                axis=AX.X,
            )
            nc.vector.reduce_sum(
                p1f[:, k0:k1],
                p1s.rearrange("p (t d) -> p t d", d=_D),
                axis=AX.X,
            )

        pending = None  # (ga, k) for the lagged p2 tail
        with nc.allow_low_precision("fp16 fold chains; final accums are f32"):
            for k in range(T):
                xt = xp.tile([_P, _FD], f16, tag="xt")
                nc.sync.dma_start(xt[:], xv_dram[k])

                # --- p1 first fold: two bulk SWDGE transfers (copy + accum)
                fa = fap.tile([_P, H], f16, tag="fa")
                if dma_fold:
                    nc.gpsimd.dma_start(fa[:], xt[:, :H])
                    nc.gpsimd.dma_start(
                        fa[:], xt[:, H:], accum_op=OP.add
                    )
                else:
                    nc.vector.tensor_add(fa[:], xt[:, :H], xt[:, H:])
                chain_tail(fa, p1b, k)

                # --- square (fp16 out)
                sq = sqp.tile([_P, _FD], f16, tag="sq")
                eng = sq_engine[k % len(sq_engine)]
                if eng == "a":
                    nc.scalar.activation(sq[:], xt[:], AF.Square)
                elif eng == "d":
                    nc.vector.tensor_mul(sq[:], xt[:], xt[:])
                else:
                    nc.gpsimd.tensor_mul(sq[:], xt[:], xt[:])

                # --- p2 first fold on GPSIMD
                ga = gap.tile([_P, H], f16, tag="ga")
                nc.gpsimd.tensor_add(ga[:], sq[:, :H], sq[:, H:])

                # --- lagged p2 DVE tail (previous tile) to avoid stalls
                if pending is not None:
                    chain_tail2(pending[0], p2b, pending[1], gbp, gcp)
                pending = (ga, k)

                # --- sin pass on ACT, dead output to PSUM, accum to sa1
                nc.scalar.activation(
                    sindead[:], xt[:], AF.Sin, scale=0.125,
                    accum_out=sa1[:, k:k + 1],
                )

                if k == T - 2:
                    emit_epi(0, T - 2)

            chain_tail2(pending[0], p2b, pending[1], gbp, gcp)
            emit_epi(T - 2, T)

            # out = eacc + 128 p1f - 1024 S1
            nc.vector.scalar_tensor_tensor(
                dq[:], p1f[:], 128.0, eacc[:], OP.mult, OP.add
            )
            nc.vector.scalar_tensor_tensor(
                out8[:], sa1[:], -1024.0, dq[:], OP.mult, OP.add
            )
        nc.sync.dma_start(y_ext[:], out8[:])

    nc.compile()
    return nc


_nc_cache = {}


def _get_nc():
    key = (_BP, _SQ_ENGINE, _DMA_FOLD)
    if key not in _nc_cache:
        _nc_cache[key] = build_nc(_BP, _SQ_ENGINE, _DMA_FOLD)
    return _nc_cache[key]


def _make_in_maps(x: np.ndarray) -> list:
    """Shard and cast to fp16 (native [b, F, D] layout; no transpose)."""
    xt = x.reshape(_NCORES, _BP, _F, _D).astype(np.float16)
    return [{"x": xt[c]} for c in range(_NCORES)]


def kernel(x: np.ndarray) -> np.ndarray:
    from concourse.bass_utils import run_bass_kernel_spmd

    x = np.ascontiguousarray(np.asarray(x, dtype=np.float32))
    assert x.shape == (_B, _F, _D), x.shape

    nc = _get_nc()
    in_maps = _make_in_maps(x)
    res = run_bass_kernel_spmd(nc, in_maps, core_ids=list(range(_NCORES)))
    outs = []
    for c in range(_NCORES):
        o = res.results[c]["out"]  # [128, T]; o[p, t] = y[t*128 + p]
        outs.append(np.asarray(o).T.reshape(-1))
    return np.concatenate(outs).reshape(_B, 1).astype(np.float32)


# revision 7
# speedup vs baseline: 1.2324x; 1.2324x over previous
"""ANOVA-kernel (order 3) Trainium2 Bass kernel.

Reference computes, per batch b: sum_d e3(x[b, :, d]) where e3 is the 3rd
elementary symmetric polynomial over the F=64 fields. Newton's identities:

    e3 = (p1^3 - 3 p1 p2 + 2 p3) / 6,   p_k[b, d] = sum_f x[b, f, d]^k

so the sequential DP scan becomes power-sum reductions.

The kernel streams x as fp16 (host-side cast; quantization contributes
~1.5e-3 norm-rel vs the 2e-2 tolerance), halving HBM traffic and enabling
the DVE 16-bit 2x mode. The input keeps its native [b, F, D] layout with
d contiguous, so every pairwise f-fold is an add of two CONTIGUOUS tile
halves: 2x-packed on DVE, bulk-transferable by DMA.

Per [128 x 4096] tile (batch on partitions, free = (f, d)):
  - p1: first f-fold runs on the DMA engines (software-DGE copy + accum
    add — the bus has ~50% slack at fp16), then two 2x folds + one small
    grouped reduce on DVE.
  - x^2 (fp16): Scalar-engine Square or DVE tensor_mul (2x), per tile.
  - p2: first fold of x^2 on GPSIMD (otherwise idle), then the same DVE
    tail.
  - sum_d p3 via the sin trick: sum sin(t x) = t P1 - t^3 P3/6 + O(t^5);
    one Scalar pass per tile, dead output to PSUM (saves SBUF write
    bandwidth), free per-partition accumulate.
  - epilogue: out = sum_d (1/6) p1 (p1^2 - 3 p2) + 128 P1 - 1024 S1.

Sharding: pure data parallel over the batch dim across 8 NeuronCores.
"""

import numpy as np

_B, _F, _D = 8192, 64, 64
_NCORES = 8
_BP = _B // _NCORES  # batches per core
_P = 128             # partitions per tile
_FD = _F * _D        # free elems per batch

# square engine per tile index: 'a' = ACT, 'd' = DVE, 'g' = GPSIMD
_SQ_ENGINE = "dadadada"
# p1 first fold on the DMA engines (software DGE accum); else DVE
_DMA_FOLD = False


def build_nc(bp=_BP, sq_engine=_SQ_ENGINE, dma_fold=_DMA_FOLD):
    """Build the per-core Bass graph for bp batches.

    Inputs:  "x"   [bp, 64, 64] f16 in native (b, f, d) layout
    Outputs: "out" [128, bp/128] f32 with out[p, t] = y[t*128 + p]
    """
    from contextlib import ExitStack

    from concourse import bacc, mybir, tile

    f32 = mybir.dt.float32
    f16 = mybir.dt.float16
    AF = mybir.ActivationFunctionType
    OP = mybir.AluOpType
    AX = mybir.AxisListType

    T = bp // _P  # tiles per core
    assert bp % _P == 0
    H = _FD // 2

    nc = bacc.Bacc("TRN2", target_bir_lowering=False, debug=False)
    x_ext = nc.dram_tensor("x", [bp, _F, _D], f16, kind="ExternalInput").ap()
    y_ext = nc.dram_tensor("out", [_P, T], f32, kind="ExternalOutput").ap()

    with tile.TileContext(nc) as tc, ExitStack() as ctx:
        xp = ctx.enter_context(tc.tile_pool(name="xt", bufs=T))
        sqp = ctx.enter_context(tc.tile_pool(name="sq", bufs=3))
        fap = ctx.enter_context(tc.tile_pool(name="fa", bufs=3))
        fbp = ctx.enter_context(tc.tile_pool(name="fb", bufs=2))
        fcp = ctx.enter_context(tc.tile_pool(name="fc", bufs=2))
        gap = ctx.enter_context(tc.tile_pool(name="ga", bufs=3))
        gbp = ctx.enter_context(tc.tile_pool(name="gb", bufs=2))
        gcp = ctx.enter_context(tc.tile_pool(name="gc", bufs=2))
        pers = ctx.enter_context(tc.tile_pool(name="pers", bufs=1))

        p1b = pers.tile([_P, T * _D], f32, tag="p1b")
        p2b = pers.tile([_P, T * _D], f32, tag="p2b")
        sa1 = pers.tile([_P, T], f32, tag="sa1")
        eacc = pers.tile([_P, T], f32, tag="eacc")
        p1f = pers.tile([_P, T], f32, tag="p1f")
        dq = pers.tile([_P, T], f32, tag="dq")
        out8 = pers.tile([_P, T], f32, tag="out8")
        sindead = pers.tile([_P, _FD], f16, tag="sindead")
        r_ = pers.tile([_P, 6 * _D], f32, tag="r_")
        z_ = pers.tile([_P, 6 * _D], f32, tag="z_")
        w_ = pers.tile([_P, 6 * _D], f32, tag="w_")

        xv_dram = x_ext.rearrange("(t p) f d -> t p (f d)", p=_P)

        # warm the Sin activation table during the initial DMA wait
        warm = pers.tile([_P, 1], f32, tag="warm")
        nc.gpsimd.memset(warm[:], 0.0)
        nc.scalar.activation(warm[:], warm[:], AF.Sin, scale=0.125)

        def chain_tail(l1, big, k):
            """Two 2x folds + grouped f-reduce: l1 [P, 2048] -> big cols k."""
            fb = fbp.tile([_P, _FD // 4], f16, tag="fb")
            nc.vector.tensor_add(fb[:], l1[:, :_FD // 4], l1[:, _FD // 4:])
            fc = fcp.tile([_P, _FD // 8], f16, tag="fc")
            nc.vector.tensor_add(fc[:], fb[:, :_FD // 8], fb[:, _FD // 8:])
            # fc layout (f=8, d=64); reduce over strided f
            nc.vector.reduce_sum(
                big[:, k * _D:(k + 1) * _D],
                fc[:].rearrange("p (f d) -> p d f", f=8),
                axis=AX.X,
            )

        def chain_tail2(l1, big, k, bp_, cp_):
            fb = bp_.tile([_P, _FD // 4], f16, tag="gb")
            nc.vector.tensor_add(fb[:], l1[:, :_FD // 4], l1[:, _FD // 4:])
            fc = cp_.tile([_P, _FD // 8], f16, tag="gc")
            nc.vector.tensor_add(fc[:], fb[:, :_FD // 8], fb[:, _FD // 8:])
            nc.vector.reduce_sum(
                big[:, k * _D:(k + 1) * _D],
                fc[:].rearrange("p (f d) -> p d f", f=8),
                axis=AX.X,
            )

        def emit_epi(k0, k1):
            """eacc = sum_d (1/6) p1 (p1^2 - 3 p2); p1f = sum_d p1."""
            n = (k1 - k0) * _D
            p1s = p1b[:, k0 * _D:k1 * _D]
            p2s = p2b[:, k0 * _D:k1 * _D]
            nc.vector.tensor_mul(r_[:, :n], p1s, p1s)
            nc.vector.scalar_tensor_tensor(
                z_[:, :n], p2s, -3.0, r_[:, :n], OP.mult, OP.add
            )
            nc.vector.scalar_tensor_tensor(
                w_[:, :n], p1s, 1.0 / 6.0, z_[:, :n], OP.mult, OP.mult
            )
            nc.vector.reduce_sum(
                eacc[:, k0:k1],
                w_[:, :n].rearrange("p (t d) -> p t d", d=_D),
                axis=AX.X,
            )
            nc.vector.reduce_sum(
                p1f[:, k0:k1],
                p1s.rearrange("p (t d) -> p t d", d=_D),
                axis=AX.X,
            )

        pending = None  # (ga, k) for the lagged p2 tail
        with nc.allow_low_precision("fp16 fold chains; final accums are f32"):
            for k in range(T):
                xt = xp.tile([_P, _FD], f16, tag="xt")
                nc.sync.dma_start(xt[:], xv_dram[k])

                # --- p1 first fold: two bulk SWDGE transfers (copy + accum)
                fa = fap.tile([_P, H], f16, tag="fa")
                if dma_fold:
                    nc.gpsimd.dma_start(fa[:], xt[:, :H])
                    nc.gpsimd.dma_start(
                        fa[:], xt[:, H:], accum_op=OP.add
                    )
                else:
                    nc.vector.tensor_add(fa[:], xt[:, :H], xt[:, H:])
                chain_tail(fa, p1b, k)

                # --- square (fp16 out)
                sq = sqp.tile([_P, _FD], f16, tag="sq")
                eng = sq_engine[k % len(sq_engine)]
                if eng == "a":
                    nc.scalar.activation(sq[:], xt[:], AF.Square)
                elif eng == "d":
                    nc.vector.tensor_mul(sq[:], xt[:], xt[:])
                else:
                    nc.gpsimd.tensor_mul(sq[:], xt[:], xt[:])

                # --- p2 first fold on GPSIMD
                ga = gap.tile([_P, H], f16, tag="ga")
                nc.gpsimd.tensor_add(ga[:], sq[:, :H], sq[:, H:])

                # --- lagged p2 DVE tail (previous tile) to avoid stalls
                if pending is not None:
                    chain_tail2(pending[0], p2b, pending[1], gbp, gcp)
                pending = (ga, k)

                # --- sin pass on ACT, dead output to PSUM, accum to sa1
                nc.scalar.activation(
                    sindead[:], xt[:], AF.Sin, scale=0.125,
                    accum_out=sa1[:, k:k + 1],
                )

                if k == T - 2:
                    emit_epi(0, T - 2)

            chain_tail2(pending[0], p2b, pending[1], gbp, gcp)
            emit_epi(T - 2, T)

            # out = eacc + 128 p1f - 1024 S1
            nc.vector.scalar_tensor_tensor(
                dq[:], p1f[:], 128.0, eacc[:], OP.mult, OP.add
            )
            nc.vector.scalar_tensor_tensor(
                out8[:], sa1[:], -1024.0, dq[:], OP.mult, OP.add
            )
        nc.sync.dma_start(y_ext[:], out8[:])

    nc.compile()
    return nc


_nc_cache = {}


def _get_nc():
    key = (_BP, _SQ_ENGINE, _DMA_FOLD)
    if key not in _nc_cache:
        _nc_cache[key] = build_nc(_BP, _SQ_ENGINE, _DMA_FOLD)
    return _nc_cache[key]


def _make_in_maps(x: np.ndarray) -> list:
    """Shard and cast to fp16 (native [b, F, D] layout; no transpose)."""
    xt = x.reshape(_NCORES, _BP, _F, _D).astype(np.float16)
    return [{"x": xt[c]} for c in range(_NCORES)]


def kernel(x: np.ndarray) -> np.ndarray:
    from concourse.bass_utils import run_bass_kernel_spmd

    x = np.ascontiguousarray(np.asarray(x, dtype=np.float32))
    assert x.shape == (_B, _F, _D), x.shape

    nc = _get_nc()
    in_maps = _make_in_maps(x)
    res = run_bass_kernel_spmd(nc, in_maps, core_ids=list(range(_NCORES)))
    outs = []
    for c in range(_NCORES):
        o = res.results[c]["out"]  # [128, T]; o[p, t] = y[t*128 + p]
        outs.append(np.asarray(o).T.reshape(-1))
    return np.concatenate(outs).reshape(_B, 1).astype(np.float32)


# revision 8
# speedup vs baseline: 1.2475x; 1.0122x over previous
"""ANOVA-kernel (order 3) Trainium2 Bass kernel.

Reference computes, per batch b: sum_d e3(x[b, :, d]) where e3 is the 3rd
elementary symmetric polynomial over the F=64 fields. Newton's identities:

    e3 = (p1^3 - 3 p1 p2 + 2 p3) / 6,   p_k[b, d] = sum_f x[b, f, d]^k

so the sequential DP scan becomes power-sum reductions.

The kernel streams x as fp16 (host-side cast; quantization contributes
~1.5e-3 norm-rel vs the 2e-2 tolerance), halving HBM traffic and enabling
the DVE 16-bit 2x mode for tensor_tensor ops. Layout is [bp, D, F]
(f contiguous), so per-d f-reductions are binary FOLD CHAINS of
within-row halves — measured ~2x faster than grouped tensor_reduce on
this hardware, with every level 2x-eligible.

Per [128 x 4096] tile (batch on partitions, free = (d, f)):
  - p1 per (b, d): 6-level DVE fold chain (last level accumulates to f32)
  - x^2 (fp16): Scalar-engine Square or DVE same-AP tensor_mul (both
    contention-immune), per tile
  - p2 per (b, d): first fold on GPSIMD (otherwise idle), then the DVE
    fold-chain tail
  - sum_d p3 via the sin trick: sum sin(t x) = t P1 - t^3 P3/6 + O(t^5);
    one Scalar pass per tile with a free per-partition accumulate
  - epilogue: out = sum_d (1/6) p1 (p1^2 - 3 p2) + 128 P1 - 1024 S1

Sharding: pure data parallel over the batch dim across 8 NeuronCores.
"""

import numpy as np

_B, _F, _D = 8192, 64, 64
_NCORES = 8
_BP = _B // _NCORES  # batches per core
_P = 128             # partitions per tile
_FD = _F * _D        # free elems per batch

# square engine per tile index: 'a' = ACT, 'd' = DVE, 'g' = GPSIMD
_SQ_ENGINE = "dadadada"


def build_nc(bp=_BP, sq_engine=_SQ_ENGINE):
    """Build the per-core Bass graph for bp batches.

    Inputs:  "x"   [bp, 64, 64] f16 in (b, d, f) layout
    Outputs: "out" [128, bp/128] f32 with out[p, t] = y[t*128 + p]
    """
    from contextlib import ExitStack

    from concourse import bacc, mybir, tile

    f32 = mybir.dt.float32
    f16 = mybir.dt.float16
    AF = mybir.ActivationFunctionType
    OP = mybir.AluOpType
    AX = mybir.AxisListType

    T = bp // _P  # tiles per core
    assert bp % _P == 0

    nc = bacc.Bacc("TRN2", target_bir_lowering=False, debug=False)
    x_ext = nc.dram_tensor("x", [bp, _D, _F], f16, kind="ExternalInput").ap()
    y_ext = nc.dram_tensor("out", [_P, T], f32, kind="ExternalOutput").ap()

    with tile.TileContext(nc) as tc, ExitStack() as ctx:
        xp = ctx.enter_context(tc.tile_pool(name="xt", bufs=T))
        sqp = ctx.enter_context(tc.tile_pool(name="sq", bufs=3))
        gap = ctx.enter_context(tc.tile_pool(name="ga", bufs=3))
        fp_ = ctx.enter_context(tc.tile_pool(name="fl", bufs=2))
        pers = ctx.enter_context(tc.tile_pool(name="pers", bufs=1))

        p1b = pers.tile([_P, T * _D], f32, tag="p1b")
        p2b = pers.tile([_P, T * _D], f32, tag="p2b")
        sa1 = pers.tile([_P, T], f32, tag="sa1")
        eacc = pers.tile([_P, T], f32, tag="eacc")
        p1f = pers.tile([_P, T], f32, tag="p1f")
        dq = pers.tile([_P, T], f32, tag="dq")
        out8 = pers.tile([_P, T], f32, tag="out8")
        sindead = pers.tile([_P, _FD], f16, tag="sindead")
        r_ = pers.tile([_P, 6 * _D], f32, tag="r_")
        z_ = pers.tile([_P, 6 * _D], f32, tag="z_")
        w_ = pers.tile([_P, 6 * _D], f32, tag="w_")

        xv_dram = x_ext.rearrange("(t p) d f -> t p (d f)", p=_P)

        # warm the Sin activation table during the initial DMA wait
        warm = pers.tile([_P, 1], f32, tag="warm")
        nc.gpsimd.memset(warm[:], 0.0)
        nc.scalar.activation(warm[:], warm[:], AF.Sin, scale=0.125)

        def halves(t, w):
            v = t[:, :_D * w].rearrange("p (d f) -> p d f", f=w)
            return v[:, :, :w // 2], v[:, :, w // 2:]

        def fold_chain(src, w, big, k, tag):
            """Binary fold chain src [P, 64*w] -> big[:, k*64:(k+1)*64] f32."""
            cur = src
            while w > 2:
                nxt = fp_.tile([_P, _D * (w // 2)], f16, tag=f"{tag}{w}")
                lo, hi = halves(cur, w)
                nc.vector.tensor_add(
                    nxt[:].rearrange("p (d f) -> p d f", f=w // 2), lo, hi
                )
                cur, w = nxt, w // 2
            lo, hi = halves(cur, 2)
            nc.vector.tensor_add(big[:, k * _D:(k + 1) * _D], lo, hi)

        def emit_epi(k0, k1):
            """eacc = sum_d (1/6) p1 (p1^2 - 3 p2); p1f = sum_d p1."""
            n = (k1 - k0) * _D
            p1s = p1b[:, k0 * _D:k1 * _D]
            p2s = p2b[:, k0 * _D:k1 * _D]
            nc.vector.tensor_mul(r_[:, :n], p1s, p1s)
            nc.vector.scalar_tensor_tensor(
                z_[:, :n], p2s, -3.0, r_[:, :n], OP.mult, OP.add
            )
            nc.vector.scalar_tensor_tensor(
                w_[:, :n], p1s, 1.0 / 6.0, z_[:, :n], OP.mult, OP.mult
            )
            nc.vector.reduce_sum(
                eacc[:, k0:k1],
                w_[:, :n].rearrange("p (t d) -> p t d", d=_D),
                axis=AX.X,
            )
            nc.vector.reduce_sum(
                p1f[:, k0:k1],
                p1s.rearrange("p (t d) -> p t d", d=_D),
                axis=AX.X,
            )

        pending = None  # (ga, k) for the lagged p2 tail
        with nc.allow_low_precision("fp16 fold chains; final accums are f32"):
            for k in range(T):
                xt = xp.tile([_P, _FD], f16, tag="xt")
                nc.sync.dma_start(xt[:], xv_dram[k])

                # --- p1: full DVE fold chain
                fold_chain(xt, _F, p1b, k, "f")

                # --- square (fp16 out)
                sq = sqp.tile([_P, _FD], f16, tag="sq")
                eng = sq_engine[k % len(sq_engine)]
                if eng == "a":
                    nc.scalar.activation(sq[:], xt[:], AF.Square)
                elif eng == "d":
                    nc.vector.tensor_mul(sq[:], xt[:], xt[:])
                else:
                    nc.gpsimd.tensor_mul(sq[:], xt[:], xt[:])

                # --- p2 first fold on GPSIMD
                ga = gap.tile([_P, _FD // 2], f16, tag="ga")
                lo, hi = halves(sq, _F)
                nc.gpsimd.tensor_add(
                    ga[:].rearrange("p (d f) -> p d f", f=_F // 2), lo, hi
                )

                # --- lagged p2 DVE fold tail (previous tile)
                if pending is not None:
                    fold_chain(pending[0], _F // 2, p2b, pending[1], "g")
                pending = (ga, k)

                # --- sin pass on ACT with free per-partition accumulate
                nc.scalar.activation(
                    sindead[:], xt[:], AF.Sin, scale=0.125,
                    accum_out=sa1[:, k:k + 1],
                )

                if k == T - 2:
                    emit_epi(0, T - 2)

            fold_chain(pending[0], _F // 2, p2b, pending[1], "g")
            emit_epi(T - 2, T)

            # out = eacc + 128 p1f - 1024 S1
            nc.vector.scalar_tensor_tensor(
                dq[:], p1f[:], 128.0, eacc[:], OP.mult, OP.add
            )
            nc.vector.scalar_tensor_tensor(
                out8[:], sa1[:], -1024.0, dq[:], OP.mult, OP.add
            )
        nc.sync.dma_start(y_ext[:], out8[:])

    nc.compile()
    return nc


_nc_cache = {}


def _get_nc():
    key = (_BP, _SQ_ENGINE)
    if key not in _nc_cache:
        _nc_cache[key] = build_nc(_BP, _SQ_ENGINE)
    return _nc_cache[key]


def _make_in_maps(x: np.ndarray) -> list:
    """Shard + transpose to [bp, D, F] and cast to fp16 (host marshaling)."""
    xt = np.ascontiguousarray(
        x.reshape(_NCORES, _BP, _F, _D).transpose(0, 1, 3, 2)
    ).astype(np.float16)
    return [{"x": xt[c]} for c in range(_NCORES)]


def kernel(x: np.ndarray) -> np.ndarray:
    from concourse.bass_utils import run_bass_kernel_spmd

    x = np.ascontiguousarray(np.asarray(x, dtype=np.float32))
    assert x.shape == (_B, _F, _D), x.shape

    nc = _get_nc()
    in_maps = _make_in_maps(x)
    res = run_bass_kernel_spmd(nc, in_maps, core_ids=list(range(_NCORES)))
    outs = []
    for c in range(_NCORES):
        o = res.results[c]["out"]  # [128, T]; o[p, t] = y[t*128 + p]
        outs.append(np.asarray(o).T.reshape(-1))
    return np.concatenate(outs).reshape(_B, 1).astype(np.float32)
